# revision 1
# baseline (speedup 1.0000x reference)
"""Trainium2 Bass kernel for nn_BaselineBlock_SCA_Modulated.

Sharding: 8 cores = 2 batch x 4 D-slabs of 16 planes each. Halo planes are
staged host-side (zero planes at global D edges), so all cores run an
identical SPMD program. The 3x3x3 modulated depthwise conv is computed on
TensorE as a per-chunk chain of 15 matmuls fusing pw1 (C=64 -> DW=128) with
tap pairs (partition-stacked shifted copies of the LN1 output), plus one
K=9 matmul adding the pw1-bias boundary correction.
"""
import numpy as np
import ml_dtypes

C, DW, SD = 64, 128, 512
D, H, W = 64, 64, 64
NPL = 16              # output planes per core
NHALO = NPL + 2       # input planes incl halo
PW = 66               # padded row width (w in [-1, 64])
PSZ = PW * PW + 2     # padded plane size + 2 slack cols (zero)
HWC = H * W           # 4096
NCH = HWC // 128      # 32 transpose chunks / plane
EPS = 1e-6
bf = ml_dtypes.bfloat16

_CACHE = {}


def _build():
    import concourse.bacc as bacc
    import concourse.mybir as mybir
    import concourse.tile as tile
    from concourse.mybir import ActivationFunctionType as AF, AluOpType as ALU

    BF = mybir.dt.bfloat16
    F32 = mybir.dt.float32
    AX = mybir.AxisListType

    nc = bacc.Bacc("TRN2", target_bir_lowering=False, debug=False, num_devices=8)

    dram = {}
    def din(name, shape, dt=BF):
        dram[name] = nc.dram_tensor(name, shape, dt, kind="ExternalInput")
        return dram[name]

    inp_t = din("inp_t", [NHALO, C, HWC], BF)
    inp_f = din("inp_f", [NPL, C, HWC], F32)
    wpair_i = din("wpair", [128, 12, 128], BF)
    wsing_i = din("wsing", [64, 3, 128], BF)
    wcorr_i = din("wcorr", [9, NPL, 128], BF)
    ind_i = din("ind", [9, 3, 512], BF)
    sd_i = din("sd", [128, 1], F32)
    modb_i = din("modb", [128, 1], F32)
    w3T_i = din("w3T", [128, 64], BF)
    scawT_i = din("scawT", [128, 128], BF)
    scab_i = din("scab", [128, 1], F32)
    beta_i = din("beta", [64, 1], F32)
    b3beta_i = din("b3beta", [64, 1], F32)
    w4T_i = din("w4T", [64, 128], BF)
    b4_i = din("b4", [128, 1], F32)
    w5T_i = din("w5T", [128, 64], BF)
    gamma_i = din("gamma", [64, 1], F32)
    b5g_i = din("b5g", [64, 1], F32)
    i128_i = din("i128", [128, 128], BF)
    i64f_i = din("i64f", [64, 64], F32)
    out_d = nc.dram_tensor("out", [NPL, C, HWC], F32, kind="ExternalOutput")

    xg_scr = nc.dram_tensor("xg_scr", [NPL, 128, HWC], BF)
    cc_a = nc.dram_tensor("cc_a", [128, 1], F32)
    cc_b = nc.dram_tensor("cc_b", [128, 1], F32)

    def geom(t, p0, pn, r0, nr, c0, ncol=64):
        """interior view [pn, nr, ncol] of a padded plane tile."""
        return t[p0:p0 + pn, 0:PW * PW].rearrange(
            "p (r w) -> p r w", w=PW)[:, r0:r0 + nr, c0:c0 + ncol]

    from contextlib import ExitStack
    with tile.TileContext(nc) as tc, ExitStack() as stk:
        cpool = stk.enter_context(tc.tile_pool(name="const", bufs=1))
        rpool = stk.enter_context(tc.tile_pool(name="ring", bufs=1))
        wpool = stk.enter_context(tc.tile_pool(name="work", bufs=2))
        xgp = stk.enter_context(tc.tile_pool(name="xg", bufs=2))
        p2p = stk.enter_context(tc.tile_pool(name="p2", bufs=1))
        psA = stk.enter_context(tc.tile_pool(name="psA", bufs=2, space="PSUM"))
        psB = stk.enter_context(tc.tile_pool(name="psB", bufs=2, space="PSUM"))

        def const(name, shape, dt):
            t = cpool.tile(shape, dt, tag=name, name=name)
            nc.sync.dma_start(t[:], dram[name][:])
            return t

        wp = const("wpair", [128, 12, 128], BF)
        ws = const("wsing", [64, 3, 128], BF)
        wc = const("wcorr", [9, NPL, 128], BF)
        ind = const("ind", [9, 3, 512], BF)
        sd = const("sd", [128, 1], F32)
        modb = const("modb", [128, 1], F32)
        w3T = const("w3T", [128, 64], BF)
        scawT = const("scawT", [128, 128], BF)
        scab = const("scab", [128, 1], F32)
        beta = const("beta", [64, 1], F32)
        b3beta = const("b3beta", [64, 1], F32)
        w4T = const("w4T", [64, 128], BF)
        b4 = const("b4", [128, 1], F32)
        w5T = const("w5T", [128, 64], BF)
        gamma = const("gamma", [64, 1], F32)
        b5g = const("b5g", [64, 1], F32)
        i128 = const("i128", [128, 128], BF)
        i64f = const("i64f", [64, 64], F32)

        pools = cpool.tile([128, NPL * 8], F32, tag="pools")
        w3Tp = cpool.tile([128, 64], BF, tag="w3Tp")

        NS = 4
        t1s = [rpool.tile([128, PSZ], BF, tag=f"t1_{i}", name=f"t1_{i}") for i in range(NS)]
        ths = [rpool.tile([128, PSZ], BF, tag=f"th_{i}", name=f"th_{i}") for i in range(NS)]
        for i in range(NS):
            nc.gpsimd.memset(t1s[i][:], 0.0)
            nc.gpsimd.memset(ths[i][:], 0.0)

        # ---------------- PASS 1 ----------------
        def ln1_plane(p):
            slot = p % NS
            t1, th = t1s[slot], ths[slot]
            xT = wpool.tile([128, NCH, 64], BF, tag="xT")
            nc.sync.dma_start_transpose(xT[:], inp_t[p])
            sq = wpool.tile([128, NCH, 64], BF, tag="sq", bufs=1)
            nc.vector.tensor_mul(sq[:], xT[:], xT[:])
            msum = wpool.tile([128, NCH], F32, tag="msum")
            qsum = wpool.tile([128, NCH], F32, tag="qsum")
            nc.vector.tensor_reduce(msum[:], xT[:], axis=AX.X, op=ALU.add)
            nc.vector.tensor_reduce(qsum[:], sq[:], axis=AX.X, op=ALU.add)
            t1v = wpool.tile([128, NCH], F32, tag="t1v")
            nc.vector.tensor_mul(t1v[:], msum[:], msum[:])
            t3v = wpool.tile([128, NCH], F32, tag="t3v")
            nc.vector.tensor_scalar_mul(t3v[:], qsum[:], 1.0 / 63.0)
            var = wpool.tile([128, NCH], F32, tag="var")
            nc.vector.scalar_tensor_tensor(
                var[:], t1v[:], -1.0 / (64.0 * 63.0), t3v[:],
                op0=ALU.mult, op1=ALU.add)
            sv = wpool.tile([128, NCH], F32, tag="sv")
            nc.scalar.activation(sv[:], var[:], AF.Sqrt)
            nc.vector.tensor_scalar_add(sv[:], sv[:], EPS)
            rv = wpool.tile([128, NCH], F32, tag="rv")
            nc.vector.reciprocal(rv[:], sv[:])
            mrv = wpool.tile([128, NCH], F32, tag="mrv")
            nc.vector.scalar_tensor_tensor(
                mrv[:], msum[:], 1.0 / 64.0, rv[:], op0=ALU.mult, op1=ALU.mult)
            xln = xT
            rvv = rv[:].unsqueeze(2).broadcast_to([128, NCH, 64])
            mrvv = mrv[:].unsqueeze(2).broadcast_to([128, NCH, 64])
            nc.vector.tensor_mul(xln[:], xT[:], rvv)
            nc.vector.tensor_sub(xln[:], xln[:], mrvv)
            for q in range(2):
                pst = psA.tile([64, 2048], BF, tag="tr")
                for g in range(16):
                    nc.tensor.transpose(
                        pst[:, g * 128:(g + 1) * 128], xln[:, 16 * q + g, :],
                        i128[:])
                nc.vector.tensor_copy(
                    geom(t1, 0, 64, 1 + 32 * q, 32, 1),
                    pst[:].rearrange("p (r w) -> p r w", w=64))
            nc.sync.dma_start(t1[64:128, 0:PSZ - 2], t1[0:64, 2:PSZ])
            nc.sync.dma_start(th[0:64, 0:PSZ - 132], t1[0:64, 132:PSZ])
            nc.sync.dma_start(th[64:128, :], t1[0:64, :])

        def conv_plane(d):
            slots = [t1s[(d + kd) % NS] for kd in range(3)]
            hslots = [ths[(d + kd) % NS] for kd in range(3)]
            xg = xgp.tile([128, HWC], BF, tag="xg")
            for cb in range(8):
                ps = psB.tile([128, 512], F32, tag="conv")
                for kd in range(3):
                    for kh in range(3):
                        nc.tensor.matmul(
                            ps[:], wp[:, kd * 3 + kh, :],
                            geom(slots[kd], 0, 128, 8 * cb + kh, 8, 0),
                            start=(kd == 0 and kh == 0), stop=False)
                for kd in range(3):
                    nc.tensor.matmul(
                        ps[:], wp[:, 9 + kd, :],
                        geom(hslots[kd], 0, 128, 8 * cb, 8, 1),
                        start=False, stop=False)
                for kd in range(3):
                    nc.tensor.matmul(
                        ps[:], ws[:, kd, :],
                        geom(slots[kd], 0, 64, 8 * cb + 1, 8, 1),
                        start=False, stop=False)
                pat = 0 if cb == 0 else (2 if cb == 7 else 1)
                nc.tensor.matmul(
                    ps[:], wc[:, d, :], ind[:, pat, :],
                    start=False, stop=True)
                nc.scalar.activation(
                    xg[:, cb * 512:(cb + 1) * 512], ps[:], AF.Gelu,
                    bias=modb[:], scale=sd[:],
                    accum_out=pools[:, d * 8 + cb:d * 8 + cb + 1])
            nc.sync.dma_start(xg_scr[d], xg[:])

        for p in range(NHALO):
            ln1_plane(p)
            if p >= 2:
                conv_plane(p - 2)

        # ---------------- pooled -> gate ----------------
        pooled = cpool.tile([128, 1], F32, tag="pooled")
        nc.vector.tensor_reduce(pooled[:], pools[:], axis=AX.X, op=ALU.add)
        nc.sync.dma_start(cc_a[:], pooled[:])
        nc.gpsimd.collective_compute(
            "AllReduce", ALU.add,
            replica_groups=[[0, 1, 2, 3], [4, 5, 6, 7]],
            ins=[cc_a[:]], outs=[cc_b[:]])
        pooled2f = cpool.tile([128, 1], F32, tag="pooled2f", name="pooled2f")
        nc.sync.dma_start(pooled2f[:], cc_b[:])
        pooled2 = cpool.tile([128, 1], BF, tag="pooled2", name="pooled2")
        nc.vector.tensor_copy(pooled2[:], pooled2f[:])
        psg = psB.tile([128, 1], F32, tag="mm")
        nc.tensor.matmul(psg[:], scawT[:], pooled2[:], start=True, stop=True)
        gate = cpool.tile([128, 1], F32, tag="gatev")
        nc.scalar.activation(gate[:], psg[:], AF.Identity, bias=scab[:])
        nc.vector.tensor_scalar_mul(w3Tp[:], w3T[:], gate[:])

        # ---------------- PASS 2 ----------------
        for d in range(NPL):
            xgt = p2p.tile([128, HWC], BF, tag="xg2")
            nc.sync.dma_start(xgt[:], xg_scr[d])
            y = p2p.tile([64, HWC], F32, tag="y", bufs=2)
            for hf in range(2):
                ifp = p2p.tile([64, 2048], F32, tag="ifp")
                nc.sync.dma_start(ifp[:], inp_f[d][:, hf * 2048:(hf + 1) * 2048])
                for cq in range(4):
                    cb = 4 * hf + cq
                    sl = slice(cb * 512, (cb + 1) * 512)
                    ps3 = psB.tile([64, 512], F32, tag="mm")
                    nc.tensor.matmul(ps3[:], w3Tp[:], xgt[:, sl],
                                     start=True, stop=True)
                    nc.vector.affine_then_add(
                        y[:, sl], ps3[:], ifp[:, cq * 512:(cq + 1) * 512],
                        scale=beta[:], bias=b3beta[:])
            # LN2 (transposed stats)
            yTs = p2p.tile([128, NCH, 64], BF, tag="yTs")
            for hf in range(2):
                psT = psA.tile([128, 1024], F32, tag="tr")
                for g in range(16):
                    cg = 16 * hf + g
                    nc.tensor.transpose(
                        psT[:, g * 64:(g + 1) * 64],
                        y[:, cg * 128:(cg + 1) * 128], i64f[:])
                nc.vector.tensor_copy(
                    yTs[:, 16 * hf:16 * (hf + 1), :],
                    psT[:].rearrange("p (g c) -> p g c", c=64))
            sq2 = wpool.tile([128, NCH, 64], BF, tag="sq2", bufs=1)
            nc.vector.tensor_mul(sq2[:], yTs[:], yTs[:])
            ms2 = wpool.tile([128, NCH], F32, tag="ms2")
            qs2 = wpool.tile([128, NCH], F32, tag="qs2")
            nc.vector.tensor_reduce(ms2[:], yTs[:], axis=AX.X, op=ALU.add)
            nc.vector.tensor_reduce(qs2[:], sq2[:], axis=AX.X, op=ALU.add)
            t1v2 = wpool.tile([128, NCH], F32, tag="t1v2")
            nc.vector.tensor_mul(t1v2[:], ms2[:], ms2[:])
            t3v2 = wpool.tile([128, NCH], F32, tag="t3v2")
            nc.vector.tensor_scalar_mul(t3v2[:], qs2[:], 1.0 / 63.0)
            var2 = wpool.tile([128, NCH], F32, tag="var2")
            nc.vector.scalar_tensor_tensor(
                var2[:], t1v2[:], -1.0 / (64.0 * 63.0), t3v2[:],
                op0=ALU.mult, op1=ALU.add)
            sv2 = wpool.tile([128, NCH], F32, tag="sv2")
            nc.scalar.activation(sv2[:], var2[:], AF.Sqrt)
            nc.vector.tensor_scalar_add(sv2[:], sv2[:], EPS)
            rv2 = wpool.tile([128, NCH], F32, tag="rv2")
            nc.vector.reciprocal(rv2[:], sv2[:])
            mrv2 = wpool.tile([128, NCH], F32, tag="mrv2")
            nc.vector.scalar_tensor_tensor(
                mrv2[:], ms2[:], 1.0 / 64.0, rv2[:], op0=ALU.mult, op1=ALU.mult)
            xl2 = yTs
            rvv2 = rv2[:].unsqueeze(2).broadcast_to([128, NCH, 64])
            mrvv2 = mrv2[:].unsqueeze(2).broadcast_to([128, NCH, 64])
            nc.vector.tensor_mul(xl2[:], yTs[:], rvv2)
            nc.vector.tensor_sub(xl2[:], xl2[:], mrvv2)
            xln2 = p2p.tile([64, HWC], BF, tag="xln2")
            for hf in range(2):
                psb = psA.tile([64, 2048], BF, tag="tr")
                for g in range(16):
                    nc.tensor.transpose(
                        psb[:, g * 128:(g + 1) * 128], xl2[:, 16 * hf + g, :],
                        i128[:])
                nc.vector.tensor_copy(
                    xln2[:, hf * 2048:(hf + 1) * 2048], psb[:])
            for hf in range(2):
                outp = p2p.tile([64, 2048], F32, tag="outp")
                xg2 = p2p.tile([128, 2048], BF, tag="xg2b")
                for cq in range(4):
                    cb = 4 * hf + cq
                    sl = slice(cb * 512, (cb + 1) * 512)
                    lsl = slice(cq * 512, (cq + 1) * 512)
                    ps4 = psB.tile([128, 512], F32, tag="mm")
                    nc.tensor.matmul(ps4[:], w4T[:], xln2[:, sl],
                                     start=True, stop=True)
                    nc.scalar.activation(xg2[:, lsl], ps4[:], AF.Gelu, bias=b4[:])
                    ps5 = psB.tile([64, 512], F32, tag="mm")
                    nc.tensor.matmul(ps5[:], w5T[:], xg2[:, lsl],
                                     start=True, stop=True)
                    nc.vector.affine_then_add(
                        outp[:, lsl], ps5[:], y[:, sl],
                        scale=gamma[:], bias=b5g[:])
                nc.sync.dma_start(
                    out_d[d][:, hf * 2048:(hf + 1) * 2048], outp[:])

    nc.compile()
    return nc


def _host_prep(inputs):
    """Per-core input maps + reassembly metadata. All folds in fp32 numpy."""
    inp = np.asarray(inputs["inp"], np.float32)
    style = np.asarray(inputs["style_vector"], np.float32)
    w1 = np.asarray(inputs["w1"], np.float32)
    b1 = np.asarray(inputs["b1"], np.float32)
    mod_w = np.asarray(inputs["mod_w"], np.float32)
    mod_b = np.asarray(inputs["mod_b"], np.float32)
    style_w = np.asarray(inputs["style_w"], np.float32)
    style_b = np.asarray(inputs["style_b"], np.float32)
    sca_w = np.asarray(inputs["sca_w"], np.float32)
    sca_b = np.asarray(inputs["sca_b"], np.float32)
    w3 = np.asarray(inputs["w3"], np.float32)
    b3 = np.asarray(inputs["b3"], np.float32)
    w4 = np.asarray(inputs["w4"], np.float32)
    b4 = np.asarray(inputs["b4"], np.float32)
    w5 = np.asarray(inputs["w5"], np.float32)
    b5 = np.asarray(inputs["b5"], np.float32)
    ln1_w = np.asarray(inputs["ln1_w"], np.float32).reshape(C)
    ln2_w = np.asarray(inputs["ln2_w"], np.float32).reshape(C)
    beta = np.asarray(inputs["beta"], np.float32).reshape(C)
    gamma = np.asarray(inputs["gamma"], np.float32).reshape(C)

    # style modulation (exact, host fp32)
    s = style @ style_w.T + style_b                     # [B, DW]
    k2 = (mod_w ** 2).sum(axis=(1, 2, 3, 4))            # [DW]
    demod = 1.0 / np.sqrt(k2[None] * s * s + 1e-8)      # [B, DW]
    sdv = s * demod                                     # [B, DW]

    W1t = w1 * ln1_w[None, :]                           # [DW, C]
    wdw = mod_w[:, 0]                                   # [DW, 3,3,3]

    # 12 pair matmuls: lhsT [128, 128]: rows 0-63 tap A via x_ln, 64-127 tap B
    wpair = np.zeros((128, 12, 128), np.float32)
    for kd in range(3):
        for kh in range(3):
            i = kd * 3 + kh
            wpair[0:64, i, :] = (W1t * wdw[:, kd, kh, 0][:, None]).T
            wpair[64:128, i, :] = (W1t * wdw[:, kd, kh, 2][:, None]).T
    for kd in range(3):
        # TH tile: lower = x_ln shifted +2 rows (tap kh=+1), upper = plain
        # (tap kh=-1), both at kw=0 (index 1)
        wpair[0:64, 9 + kd, :] = (W1t * wdw[:, kd, 2, 1][:, None]).T
        wpair[64:128, 9 + kd, :] = (W1t * wdw[:, kd, 0, 1][:, None]).T
    wsing = np.zeros((64, 3, 128), np.float32)
    for kd in range(3):
        wsing[:, kd, :] = (W1t * wdw[:, kd, 1, 1][:, None]).T

    # boundary-correction coefficients (b1 * sum of valid taps)
    def S(cd, ch, cw):
        vd = {0: [1, 2], 1: [0, 1, 2], 2: [0, 1]}[cd]
        vh = {0: [1, 2], 1: [0, 1, 2], 2: [0, 1]}[ch]
        vw = {0: [1, 2], 1: [0, 1, 2], 2: [0, 1]}[cw]
        return wdw[:, vd][:, :, vh][:, :, :, vw].sum(axis=(1, 2, 3))  # [DW]

    g = np.zeros((9, 64, 64), np.float32)
    g[0] = 1.0
    g[1, 0, :] = 1.0          # h = 0
    g[2, 63, :] = 1.0         # h = 63
    g[3, :, 0] = 1.0          # w = 0
    g[4, :, 63] = 1.0         # w = 63
    g[5, 0, 0] = 1.0
    g[6, 0, 63] = 1.0
    g[7, 63, 0] = 1.0
    g[8, 63, 63] = 1.0
    ind = np.zeros((9, 3, 512), np.float32)
    ind[:, 0] = g[:, 0:8, :].reshape(9, -1)     # chunk 0 (h=0 edge)
    ind[:, 1] = g[:, 8:16, :].reshape(9, -1)    # generic middle chunk
    ind[:, 2] = g[:, 56:64, :].reshape(9, -1)   # chunk 7 (h=63 edge)

    def corr_for(dcase):
        c = np.zeros((9, 128), np.float32)
        base = S(dcase, 1, 1)
        ch0 = S(dcase, 0, 1) - base
        ch1 = S(dcase, 2, 1) - base
        cw0 = S(dcase, 1, 0) - base
        cw1 = S(dcase, 1, 2) - base
        c[0] = base
        c[1], c[2], c[3], c[4] = ch0, ch1, cw0, cw1
        c[5] = S(dcase, 0, 0) - S(dcase, 0, 1) - S(dcase, 1, 0) + base
        c[6] = S(dcase, 0, 2) - S(dcase, 0, 1) - S(dcase, 1, 2) + base
        c[7] = S(dcase, 2, 0) - S(dcase, 2, 1) - S(dcase, 1, 0) + base
        c[8] = S(dcase, 2, 2) - S(dcase, 2, 1) - S(dcase, 1, 2) + base
        return c * b1[None, :]

    corr_tab = {c: corr_for(c) for c in (0, 1, 2)}

    common = dict(
        wpair=wpair.astype(bf), wsing=wsing.astype(bf),
        ind=ind.astype(bf),
        modb=mod_b.reshape(128, 1).astype(np.float32),
        w3T=w3.T.astype(bf),
        scawT=(sca_w.T / float(D * H * W)).astype(bf),
        scab=sca_b.reshape(128, 1).astype(np.float32),
        beta=beta.reshape(64, 1), b3beta=(b3 * beta).reshape(64, 1),
        w4T=(w4 * ln2_w[None, :]).T.astype(bf),
        b4=b4.reshape(128, 1).astype(np.float32),
        w5T=w5.T.astype(bf),
        gamma=gamma.reshape(64, 1), b5g=(b5 * gamma).reshape(64, 1),
        i128=np.eye(128, dtype=np.float32).astype(bf),
        i64f=np.eye(64, dtype=np.float32),
    )

    in_maps = []
    for k in range(8):
        b, d0 = k // 4, (k % 4) * NPL
        ip = inp[b]                                     # [C, D, H, W]
        halo = np.zeros((NHALO, C, HWC), np.float32)
        lo, hi = max(d0 - 1, 0), min(d0 + NPL + 1, D)
        halo[lo - (d0 - 1):hi - (d0 - 1)] = (
            ip[:, lo:hi].transpose(1, 0, 2, 3).reshape(hi - lo, C, HWC))
        wcorr = np.zeros((9, NPL, 128), np.float32)
        for i in range(NPL):
            dg = d0 + i
            dcase = 0 if dg == 0 else (2 if dg == D - 1 else 1)
            wcorr[:, i, :] = corr_tab[dcase]
        m = dict(common)
        m["inp_t"] = halo.astype(bf)
        m["inp_f"] = np.ascontiguousarray(
            ip[:, d0:d0 + NPL].transpose(1, 0, 2, 3).reshape(NPL, C, HWC))
        m["wcorr"] = wcorr.astype(bf)
        m["sd"] = sdv[b].reshape(128, 1).astype(np.float32)
        in_maps.append(m)
    return in_maps


def kernel(**inputs):
    from concourse.bass_utils import run_bass_kernel_spmd
    if "nc" not in _CACHE:
        _CACHE["nc"] = _build()
    nc = _CACHE["nc"]
    in_maps = _host_prep(inputs)
    res = run_bass_kernel_spmd(nc, in_maps, list(range(8)))
    _CACHE["last_res"] = res
    out = np.empty((2, C, D, H, W), np.float32)
    for k in range(8):
        b, d0 = k // 4, (k % 4) * NPL
        o = res.results[k]["out"]                       # [NPL, C, HWC]
        out[b, :, d0:d0 + NPL] = o.reshape(NPL, C, H, W).transpose(1, 0, 2, 3)
    return out



# revision 10
# speedup vs baseline: 1.7935x; 1.7935x over previous
"""Trainium2 Bass kernel for nn_BaselineBlock_SCA_Modulated (v2, fp8 conv).

Sharding: 8 cores = 2 batch x 4 D-slabs of 16 planes. Halo planes staged
host-side (zero at global D edges) so all cores run one SPMD program.

Main ideas vs v1:
- The fused pw1+3x3x3-depthwise conv runs as 10 fp8e4m3 DoubleRow matmuls
  per 512-position chunk (each contracting 2x128 rows at 0.5 cyc/row),
  reading R-tiles that stack two consecutive padded planes (64ch + 64ch)
  in the partition dim; the j-dim of DoubleRow pairs two (kh,kw) taps with
  an even element-stride delta. The pw1-bias boundary correction rides a
  spare j-slot against an indicator region stored beside each plane.
- LN statistics via one squared pass + two DVE reduces; rsqrt computed on
  DVE with the bit-trick + one Newton step (no act-table switching: the
  Act engine only ever uses the gelu table).
- All layout flips are DMA transposes (SBUF<->SBUF bf16); padding+fp8
  conversion of conv input happens on gpsimd (Pool).
- xg (post-gelu conv output) stays SBUF-resident in fp8; w3 is a plain
  fp8 matmul; pass-2 psum evacuations are split between DVE (ISA affine)
  and Act (identity-matmul accumulate + scaled copy) to balance engines.
"""
import numpy as np
import ml_dtypes

C, DW, SD = 64, 128, 512
D, H, W = 64, 64, 64
NPL = 16              # output planes per core
NHALO = NPL + 2
PW = 66               # padded row width
PLA = PW * PW         # padded plane area = 4356
REGB = PLA + 4        # region base (4360, even)
REGS = 544            # per-pattern spacing
RW = REGB + 3 * REGS  # R tile width = 5992
HWC = H * W
EPS = 1e-6
MAGIC = 0x5F3759DF
bf = ml_dtypes.bfloat16
f8 = ml_dtypes.float8_e4m3fn

# which pass-2 chunk slots evacuate via DVE ISA-affine (rest via Act)
Y_DVE = (0, 1, 2, 3, 4)     # y-form: 5 chunks on DVE
O_DVE = ()                  # final: all 8 on Act

_CACHE = {}


def _build():
    import concourse.bacc as bacc
    import concourse.mybir as mybir
    import concourse.tile as tile
    from concourse.ap import AP as BassAP
    from concourse.mybir import ActivationFunctionType as AF, AluOpType as ALU

    BF = mybir.dt.bfloat16
    F32 = mybir.dt.float32
    F8 = mybir.dt.float8e4
    I32 = mybir.dt.int32
    AX = mybir.AxisListType
    DR = mybir.MatmulPerfMode.DoubleRow

    nc = bacc.Bacc("TRN2", target_bir_lowering=False, debug=False, num_devices=8)

    dram = {}
    def din(name, shape, dt):
        dram[name] = nc.dram_tensor(name, shape, dt, kind="ExternalInput")
        return dram[name]

    import os
    inp_t = din("inp_t", [NHALO, C, HWC], BF)
    inp_b = din("inp_b", [NPL, C, HWC], BF)
    wa_i = din("wa", [128, 4, 2, 128], F8)
    wa4_i = din("wa4", [128, NPL, 2, 128], F8)
    wb_i = din("wb", [128, 4, 2, 128], F8)
    wb4_i = din("wb4", [128, 2, 128], F8)
    reg_i = din("reg", [9, 3 * REGS], F8)
    sd8_i = din("sd8", [128, 1], F32)
    modb_i = din("modb2", [128, 1], F32)
    w3tp_i = din("w3tp", [128, 64], BF)
    identA_i = din("identA", [65, 64], BF)
    identB_i = din("identB", [65, 64], BF)
    beta3_i = din("beta3", [64, 1], F32)
    b3b_i = din("b3b", [64, 1], F32)
    w4T_i = din("w4T", [64, 128], BF)
    b4_i = din("b4", [128, 1], F32)
    w5gT_i = din("w5gT", [128, 64], BF)
    b5g_i = din("b5g", [64, 1], F32)
    scawT_i = din("scawT", [128, 128], BF)
    scab_i = din("scab", [128, 1], F32)
    out_d = nc.dram_tensor("out", [NPL, C, HWC], BF, kind="ExternalOutput")
    import os
    DBG = bool(int(os.environ.get("NKDEBUG", "0")))
    if DBG:
        dbg_xg = nc.dram_tensor("dbg_xg", [128, NPL * HWC], F8,
                                kind="ExternalOutput")
        dbg_xln = nc.dram_tensor("dbg_xln", [64, HWC], BF,
                                 kind="ExternalOutput")
        dbg_R = nc.dram_tensor("dbg_R", [128, RW], F8, kind="ExternalOutput")
        dbg_y = nc.dram_tensor("dbg_y", [65, HWC], BF, kind="ExternalOutput")
        dbg_ps = nc.dram_tensor("dbg_ps", [128, 512], F32,
                                kind="ExternalOutput")

    cc_a = nc.dram_tensor("cc_a", [128, 1], F32)
    cc_b = nc.dram_tensor("cc_b", [128, 1], F32)

    from contextlib import ExitStack
    LIN = bool(int(os.environ.get("NKLIN", "0")))
    with tile.TileContext(nc, linearize=LIN) as tc, ExitStack() as stk:
        cpool = stk.enter_context(tc.tile_pool(name="const", bufs=1))
        rpool = stk.enter_context(tc.tile_pool(name="ring", bufs=1))
        wpool = stk.enter_context(tc.tile_pool(name="work", bufs=2))
        psA = stk.enter_context(tc.tile_pool(name="psA", bufs=2, space="PSUM"))
        psB = stk.enter_context(tc.tile_pool(name="psB", bufs=2, space="PSUM"))

        def const(name, shape, dt):
            t = cpool.tile(shape, dt, tag=name, name=name)
            nc.sync.dma_start(t[:], dram[name][:])
            return t

        wa = const("wa", [128, 4, 2, 128], F8)
        wa4 = const("wa4", [128, NPL, 2, 128], F8)
        wb = const("wb", [128, 4, 2, 128], F8)
        wb4 = const("wb4", [128, 2, 128], F8)
        sd8 = const("sd8", [128, 1], F32)
        modb2 = const("modb2", [128, 1], F32)
        w3tp = const("w3tp", [128, 64], BF)
        identA = const("identA", [65, 64], BF)
        identB = const("identB", [65, 64], BF)
        beta3 = const("beta3", [64, 1], F32)
        b3b = const("b3b", [64, 1], F32)
        w4T = const("w4T", [64, 128], BF)
        b4 = const("b4", [128, 1], F32)
        w5gT = const("w5gT", [128, 64], BF)
        b5g = const("b5g", [64, 1], F32)
        scawT = const("scawT", [128, 128], BF)
        scab = const("scab", [128, 1], F32)

        xg = cpool.tile([128, NPL * HWC], F8, tag="xg", name="xg")
        pools = cpool.tile([128, NPL * 8], F32, tag="pools")
        w3g8 = cpool.tile([128, 64], F8, tag="w3g8")

        NS = 4
        Rs = [rpool.tile([128, RW], F8, tag=f"R{i}", name=f"R{i}")
              for i in range(NS)]
        for i in range(NS):
            nc.gpsimd.memset(Rs[i][:], 0.0)
            nc.sync.dma_start(Rs[i][0:9, REGB:RW], reg_i[:])
        ys = [rpool.tile([65, HWC], BF, tag=f"y{i}", name=f"y{i}")
              for i in range(2)]
        ifps = [rpool.tile([65, HWC], BF, tag=f"ifp{i}", name=f"ifp{i}")
                for i in range(2)]
        for i in range(2):
            nc.gpsimd.memset(ys[i][64:65, :], 1.0)
            nc.gpsimd.memset(ifps[i][64:65, :], 1.0)
        for _ in range(2):
            t = wpool.tile([128, 32, 128], BF, tag="xln")
            nc.gpsimd.memset(t[:], 0.0)
            t2 = wpool.tile([128, 32, 128], BF, tag="xln2")
            nc.gpsimd.memset(t2[:], 0.0)

        def rsqrt_dve(out_f32, a_f32, shape, scale):
            """out = scale / sqrt(a), one Newton step. Tiles [128, n] f32."""
            n = shape[1]
            y0i = wpool.tile(shape, I32, tag="y0i")
            nc.vector.tensor_scalar(y0i[:], a_f32.bitcast(I32), 1, None,
                                    op0=ALU.arith_shift_right)
            nc.vector.tensor_scalar(y0i[:], y0i[:], MAGIC, -1,
                                    op0=ALU.subtract, op1=ALU.mult)
            y0 = y0i[:].bitcast(F32)
            y2 = wpool.tile(shape, F32, tag="y2")
            nc.vector.tensor_mul(y2[:], y0, y0)
            nc.vector.tensor_mul(y2[:], y2[:], a_f32)
            nc.vector.tensor_scalar(y2[:], y2[:], -0.5, 1.5,
                                    op0=ALU.mult, op1=ALU.add)
            nc.vector.scalar_tensor_tensor(out_f32, y0, scale, y2[:],
                                           op0=ALU.mult, op1=ALU.mult)

        def interleave2(dst, src, n):
            """dst[128, 2n] <- src[128, n] interleaved (each value twice)."""
            v = dst[:, 0:2 * n:2]
            nc.vector.tensor_copy(v, src[:])
            v2 = dst[:, 1:2 * n:2]
            nc.vector.tensor_copy(v2, src[:])

        def pair4(t, n):
            """[128, n*64] tile viewed as [128, n, 32, 2] (G, c', b)."""
            v = t[:, 0:1]
            return BassAP(v.tensor, v.offset,
                          [list(v.ap[0]), [64, n], [2, 32], [1, 2]])

        def bcast4(t2, n):
            """interleaved [128, 2n] viewed as [128, n, 32, 2] broadcast."""
            v = t2[:, 0:1]
            return BassAP(v.tensor, v.offset,
                          [list(v.ap[0]), [2, n], [0, 32], [1, 2]])

        def ln_stats(xT, sq_tile, r2, mr2, ng):
            """xT [128, ng, 64] bf16 -> r2/mr2 [128, 2*ng] f32 interleaved."""
            ms = wpool.tile([128, ng], F32, tag="ms")
            qs = wpool.tile([128, ng], F32, tag="qs")
            nc.vector.tensor_reduce(ms[:], xT, axis=AX.X, op=ALU.add)
            nc.vector.tensor_reduce(qs[:], sq_tile, axis=AX.X, op=ALU.add)
            t1 = wpool.tile([128, ng], F32, tag="t1v")
            nc.vector.tensor_mul(t1[:], ms[:], ms[:])
            av = wpool.tile([128, ng], F32, tag="av")
            nc.vector.scalar_tensor_tensor(av[:], t1[:], -1.0 / 64.0, qs[:],
                                           op0=ALU.mult, op1=ALU.add)
            nc.vector.tensor_scalar_max(av[:], av[:], 1e-20)
            rv = wpool.tile([128, ng], F32, tag="rv")
            rsqrt_dve(rv[:], av[:], [128, ng], float(np.sqrt(63.0)))
            mr = wpool.tile([128, ng], F32, tag="mrv")
            nc.vector.scalar_tensor_tensor(mr[:], ms[:], 1.0 / 64.0, rv[:],
                                           op0=ALU.mult, op1=ALU.mult)
            interleave2(r2, rv, ng)
            interleave2(mr2, mr, ng)

        # ---------------- PASS 1 ----------------
        def wideview(t, inner=64):
            # [128, 32, inner] strided view of a [128, 32, 128] wide tile
            v = t[:, 0:1]
            return BassAP(v.tensor, v.offset,
                          [list(v.ap[0]), [128, 32], [1, inner]])

        def pair4w(t):
            v = t[:, 0:1]
            return BassAP(v.tensor, v.offset,
                          [list(v.ap[0]), [128, 32], [2, 32], [1, 2]])

        def ln1_plane(p):
            xT = wpool.tile([128, 32, 64], BF, tag="xT")
            nc.sync.dma_start_transpose(xT[:], inp_t[p])
            xln = wpool.tile([128, 32, 128], BF, tag="xln")
            sqv = wideview(xln)
            nc.vector.tensor_tensor(sqv, xT[:], xT[:], op=ALU.mult)
            r2 = wpool.tile([128, 64], BF, tag="r2")
            mr2 = wpool.tile([128, 64], BF, tag="mr2")
            ln_stats(xT[:], sqv, r2, mr2, 32)
            nc.vector.tensor_tensor(pair4w(xln), pair4(xT, 32),
                                    bcast4(r2, 32), op=ALU.mult)
            nc.vector.tensor_tensor(pair4w(xln), pair4w(xln),
                                    bcast4(mr2, 32), op=ALU.subtract)
            s_t = wpool.tile([128, 32, 128], BF, tag="scm", bufs=1)
            nc.sync.dma_start_transpose(
                s_t[:], xln[:].rearrange("p a b -> p (a b)"))
            # pad + fp8 into R[p%NS] lower: two h-parity pieces
            Rt = Rs[p % NS]
            for ph in range(2):
                dst = Rt[0:64, (ph + 1) * PW + 1:(ph + 1) * PW + 2]
                dst3 = BassAP(dst.tensor, dst.offset,
                              [list(dst.ap[0]), [2 * PW, 32], [1, 64]])
                sv = s_t[0:64, 0:1]
                src3 = BassAP(sv.tensor, sv.offset + 64 * ph,
                              [list(sv.ap[0]), [128, 32], [1, 64]])
                nc.gpsimd.tensor_copy(dst3, src3)
            if DBG and p == 8:
                nc.sync.dma_start(
                    dbg_xln[:],
                    s_t[0:64, :].rearrange("p a b -> p (a b)"))
            # replicate into R[(p-1)%NS] upper
            Rp = Rs[(p - 1) % NS]
            nc.sync.dma_start(Rp[64:128, 0:PLA], Rt[0:64, 0:PLA])

        def conv_plane(d):
            TA = Rs[d % NS]
            TB = Rs[(d + 1) % NS]
            for cb in range(8):
                pat = 0 if cb == 0 else (2 if cb == 7 else 1)
                dcor = (REGB + REGS * pat + 1) - ((8 * cb + 1) * PW + 1)
                ps = psA.tile([128, 512], F32, tag="mm128")
                mlist = []
                for (T, WT, w4t) in ((TA, wa, wa4[:, d]), (TB, wb, wb4[:])):
                    bases = [8 * cb * PW, (8 * cb + 1) * PW,
                             (8 * cb + 2) * PW, 8 * cb * PW + 1,
                             (8 * cb + 1) * PW + 1]
                    dlts = [2, 2, 2, 132,
                            dcor if T is TA else 2]
                    for i in range(5):
                        v = T[:, bases[i]:bases[i] + 1]
                        rhs = BassAP(v.tensor, v.offset,
                                     [list(v.ap[0]), [dlts[i], 2],
                                      [PW, 8], [1, 64]])
                        lhs = WT[:, i] if i < 4 else w4t
                        mlist.append((lhs, rhs))
                for i, (lhs, rhs) in enumerate(mlist):
                    nc.tensor.matmul(ps[:], lhs, rhs,
                                     start=(i == 0), stop=(i == 9),
                                     perf_mode=DR)
                if DBG and d == 1 and cb == 3:
                    dpst = cpool.tile([128, 512], F32, tag="dbgps")
                    nc.vector.tensor_copy(dpst[:], ps[:])
                    nc.sync.dma_start(dbg_ps[:], dpst[:])
                col = d * 8 + cb
                nc.scalar.activation(
                    xg[:, col * 512:(col + 1) * 512], ps[:], AF.Gelu,
                    bias=modb2[:], scale=sd8[:],
                    accum_out=pools[:, col:col + 1])

        for p in range(NHALO):
            ln1_plane(p)
            if p >= 2:
                conv_plane(p - 2)

        if DBG:
            nc.sync.dma_start(dbg_xg[:], xg[:])
            nc.sync.dma_start(dbg_R[:], Rs[2][:])

        # ---------------- pooled -> gate ----------------
        pooled = cpool.tile([128, 1], F32, tag="pooled")
        nc.vector.tensor_reduce(pooled[:], pools[:], axis=AX.X, op=ALU.add)
        nc.sync.dma_start(cc_a[:], pooled[:])
        nc.gpsimd.collective_compute(
            "AllReduce", ALU.add,
            replica_groups=[[0, 1, 2, 3], [4, 5, 6, 7]],
            ins=[cc_a[:]], outs=[cc_b[:]])
        pooled2f = cpool.tile([128, 1], F32, tag="pooled2f", name="pooled2f")
        nc.sync.dma_start(pooled2f[:], cc_b[:])
        pooled2 = cpool.tile([128, 1], BF, tag="pooled2", name="pooled2")
        nc.vector.tensor_copy(pooled2[:], pooled2f[:])
        psg = psA.tile([128, 512], F32, tag="mm128")
        nc.tensor.matmul(psg[:, 0:1], scawT[:], pooled2[:], start=True,
                         stop=True)
        gate = cpool.tile([128, 1], F32, tag="gatev")
        nc.scalar.activation(gate[:], psg[:, 0:1], AF.Identity, bias=scab[:])
        w3gb = cpool.tile([128, 64], BF, tag="w3gb")
        nc.vector.tensor_scalar_mul(w3gb[:], w3tp[:], gate[:])
        nc.vector.tensor_copy(w3g8[:], w3gb[:])

        # ---------------- PASS 2 ----------------
        for d in range(NPL):
            ifp = ifps[d % 2]
            nc.sync.dma_start(ifp[0:64, :], inp_b[d])
            y = ys[d % 2]
            for cb in range(8):
                sl = slice(cb * 512, (cb + 1) * 512)
                ps3 = psB.tile([64, 512], F32, tag="mm64")
                if cb in Y_DVE:
                    nc.tensor.matmul(ps3[:], w3g8[:],
                                     xg[:, d * HWC + cb * 512:
                                        d * HWC + (cb + 1) * 512],
                                     start=True, stop=True)
                    nc.vector.affine_then_add(y[0:64, sl], ps3[:],
                                              ifp[0:64, sl],
                                              scale=beta3[:], bias=b3b[:])
                else:
                    nc.tensor.matmul(ps3[:], w3g8[:],
                                     xg[:, d * HWC + cb * 512:
                                        d * HWC + (cb + 1) * 512],
                                     start=True, stop=False)
                    nc.tensor.matmul(ps3[:], identA[:], ifp[0:65, sl],
                                     start=False, stop=True)
                    nc.scalar.activation(y[0:64, sl], ps3[:], AF.Identity,
                                         scale=beta3[:])
            if DBG and d == 0:
                nc.sync.dma_start(dbg_y[:], y[:])
            # LN2
            yT = wpool.tile([128, 32, 64], BF, tag="yT")
            nc.sync.dma_start_transpose(yT[:], y[0:64, :])
            xln2 = wpool.tile([128, 32, 128], BF, tag="xln2")
            sq2v = wideview(xln2)
            nc.scalar.activation(
                sq2v, yT[:].rearrange("p a b -> p (a b)"), AF.Square)
            r2b = wpool.tile([128, 64], BF, tag="r2b")
            mr2b = wpool.tile([128, 64], BF, tag="mr2b")
            ln_stats(yT[:], sq2v, r2b, mr2b, 32)
            nc.vector.tensor_tensor(pair4w(xln2), pair4(yT, 32),
                                    bcast4(r2b, 32), op=ALU.mult)
            nc.vector.tensor_tensor(pair4w(xln2), pair4w(xln2),
                                    bcast4(mr2b, 32), op=ALU.subtract)
            x2cm = wpool.tile([128, 32, 128], BF, tag="x2cm", bufs=1)
            nc.sync.dma_start_transpose(
                x2cm[:], xln2[:].rearrange("p a b -> p (a b)"))
            x2v = x2cm[0:64, :].rearrange("p a b -> p (a b)")
            for hf in range(2):
                outb = wpool.tile([64, 2048], BF, tag="outb")
                for cq in range(4):
                    cb = hf * 4 + cq
                    sl = slice(cb * 512, (cb + 1) * 512)
                    lsl = slice(cq * 512, (cq + 1) * 512)
                    ps4 = psA.tile([128, 512], F32, tag="mm128")
                    nc.tensor.matmul(ps4[:], w4T[:], x2v[:, sl],
                                     start=True, stop=True)
                    xg2 = wpool.tile([128, 512], BF, tag="xg2")
                    nc.scalar.activation(xg2[:], ps4[:], AF.Gelu, bias=b4[:])
                    ps5 = psB.tile([64, 512], F32, tag="mm64b")
                    if cb in O_DVE:
                        nc.tensor.matmul(ps5[:], w5gT[:], xg2[:],
                                         start=True, stop=True)
                        nc.vector.affine_then_add(outb[:, lsl], ps5[:],
                                                  y[0:64, sl],
                                                  scale=1.0, bias=b5g[:])
                    else:
                        nc.tensor.matmul(ps5[:], w5gT[:], xg2[:],
                                         start=True, stop=False)
                        nc.tensor.matmul(ps5[:], identB[:], y[0:65, sl],
                                         start=False, stop=True)
                        nc.scalar.activation(outb[:, lsl], ps5[:],
                                             AF.Identity)
                nc.sync.dma_start(
                    out_d[d][:, hf * 2048:(hf + 1) * 2048], outb[:])

    nc.compile()
    return nc


def _host_prep(inputs):
    inp = np.asarray(inputs["inp"], np.float32)
    style = np.asarray(inputs["style_vector"], np.float32)
    w1 = np.asarray(inputs["w1"], np.float32)
    b1 = np.asarray(inputs["b1"], np.float32)
    mod_w = np.asarray(inputs["mod_w"], np.float32)
    mod_b = np.asarray(inputs["mod_b"], np.float32)
    style_w = np.asarray(inputs["style_w"], np.float32)
    style_b = np.asarray(inputs["style_b"], np.float32)
    sca_w = np.asarray(inputs["sca_w"], np.float32)
    sca_b = np.asarray(inputs["sca_b"], np.float32)
    w3 = np.asarray(inputs["w3"], np.float32)
    b3 = np.asarray(inputs["b3"], np.float32)
    w4 = np.asarray(inputs["w4"], np.float32)
    b4 = np.asarray(inputs["b4"], np.float32)
    w5 = np.asarray(inputs["w5"], np.float32)
    b5 = np.asarray(inputs["b5"], np.float32)
    ln1_w = np.asarray(inputs["ln1_w"], np.float32).reshape(C)
    ln2_w = np.asarray(inputs["ln2_w"], np.float32).reshape(C)
    beta = np.asarray(inputs["beta"], np.float32).reshape(C)
    gamma = np.asarray(inputs["gamma"], np.float32).reshape(C)

    s = style @ style_w.T + style_b
    k2 = (mod_w ** 2).sum(axis=(1, 2, 3, 4))
    demod = 1.0 / np.sqrt(k2[None] * s * s + 1e-8)
    sdv = s * demod                                    # [B, DW]

    W1t = w1 * ln1_w[None, :]                          # [DW, C]
    wdw = mod_w[:, 0]                                  # [DW, 3, 3, 3]

    # per-out-channel pow2 scale for conv fp8 weights
    wmax = (np.abs(W1t).max(axis=1) * np.abs(wdw).reshape(DW, -1).max(axis=1))
    s_exp = np.floor(np.log2(16.0 / np.maximum(wmax, 1e-12)))
    s_exp = np.clip(s_exp, -20, 20)
    wsc = (2.0 ** s_exp)                               # [DW]

    def wtap(kd, kh, kw):
        # [64, 128]: lhsT rows = in-ch, cols = out-ch, scaled
        return (W1t * wdw[:, kd, kh, kw][:, None] * wsc[:, None]).T

    j_pairs = [((0, 0), (0, 2)), ((1, 0), (1, 2)), ((2, 0), (2, 2)),
               ((0, 1), (2, 1))]
    wa = np.zeros((4, 128, 2, 128), np.float32)
    wb = np.zeros((4, 128, 2, 128), np.float32)
    for i, (t0, t1) in enumerate(j_pairs):
        for j, (kh, kw) in enumerate((t0, t1)):
            wa[i, 0:64, j, :] = wtap(0, kh, kw)
            wa[i, 64:128, j, :] = wtap(1, kh, kw)
            wb[i, 64:128, j, :] = wtap(2, kh, kw)
    wb4 = np.zeros((128, 2, 128), np.float32)
    wb4[64:128, 0, :] = wtap(2, 1, 1)

    def S(cd, ch, cw):
        vd = {0: [1, 2], 1: [0, 1, 2], 2: [0, 1]}[cd]
        vh = {0: [1, 2], 1: [0, 1, 2], 2: [0, 1]}[ch]
        vw = {0: [1, 2], 1: [0, 1, 2], 2: [0, 1]}[cw]
        return wdw[:, vd][:, :, vh][:, :, :, vw].sum(axis=(1, 2, 3))

    base_mid = S(1, 1, 1)                              # sum of all taps

    def corr_for(dcase):
        c = np.zeros((9, 128), np.float32)
        base = S(dcase, 1, 1)
        c[0] = base - base_mid
        ch0 = S(dcase, 0, 1) - base
        ch1 = S(dcase, 2, 1) - base
        cw0 = S(dcase, 1, 0) - base
        cw1 = S(dcase, 1, 2) - base
        c[1], c[2], c[3], c[4] = ch0, ch1, cw0, cw1
        c[5] = S(dcase, 0, 0) - S(dcase, 0, 1) - S(dcase, 1, 0) + base
        c[6] = S(dcase, 0, 2) - S(dcase, 0, 1) - S(dcase, 1, 2) + base
        c[7] = S(dcase, 2, 0) - S(dcase, 2, 1) - S(dcase, 1, 0) + base
        c[8] = S(dcase, 2, 2) - S(dcase, 2, 1) - S(dcase, 1, 2) + base
        return c * b1[None, :]

    corr_tab = {c: corr_for(c) for c in (0, 1, 2)}

    g = np.zeros((9, 64, 64), np.float32)
    g[0] = 1.0
    g[1, 0, :] = 1.0
    g[2, 63, :] = 1.0
    g[3, :, 0] = 1.0
    g[4, :, 63] = 1.0
    g[5, 0, 0] = 1.0
    g[6, 0, 63] = 1.0
    g[7, 63, 0] = 1.0
    g[8, 63, 63] = 1.0
    hsl = {0: slice(0, 8), 1: slice(8, 16), 2: slice(56, 64)}
    reg = np.zeros((9, 3 * REGS), np.float32)
    for pat in range(3):
        blk = g[:, hsl[pat], :]                        # [9, 8, 64]
        for r in range(8):
            o = REGS * pat + 1 + r * PW
            reg[:, o:o + 64] = blk[:, r, :]

    # w3: per-out-col scale 2^u (expect gate ~O(0.25))
    w3T = w3.T                                         # [DW, C].T = [128, 64]
    u_exp = np.floor(np.log2(16.0 / np.maximum(
        np.abs(w3T).max(axis=0) * 0.25, 1e-12)))
    u_exp = np.clip(u_exp, -20, 20)
    usc = 2.0 ** u_exp                                 # [C]
    w3tp = (w3T * usc[None, :]).astype(bf)
    beta3 = (beta / usc).reshape(64, 1).astype(np.float32)
    b3b = (b3 * beta).reshape(64, 1).astype(np.float32)
    identA = np.zeros((65, 64), np.float32)
    identA[0:64] = np.diag(usc / beta)
    identA[64] = usc * b3
    identB = np.zeros((65, 64), np.float32)
    identB[0:64] = np.eye(64)
    identB[64] = b5 * gamma

    common = dict(
        wa=wa.transpose(1, 0, 2, 3).astype(f8),
        wb=wb.transpose(1, 0, 2, 3).astype(f8), wb4=wb4.astype(f8),
        reg=reg.astype(f8),
        modb2=None,  # per-core (sd-dependent)
        w3tp=w3tp,
        identA=identA.astype(bf), identB=identB.astype(bf),
        beta3=beta3, b3b=b3b,
        w4T=(w4 * ln2_w[None, :]).T.astype(bf),
        b4=b4.reshape(128, 1).astype(np.float32),
        w5gT=(w5 * gamma[:, None]).T.astype(bf),
        b5g=(b5 * gamma).reshape(64, 1).astype(np.float32),
        scawT=(sca_w.T / float(D * H * W)).astype(bf),
        scab=sca_b.reshape(128, 1).astype(np.float32),
    )

    in_maps = []
    for k in range(8):
        b, d0 = k // 4, (k % 4) * NPL
        ip = inp[b]
        halo = np.zeros((NHALO, C, HWC), np.float32)
        lo, hi = max(d0 - 1, 0), min(d0 + NPL + 1, D)
        halo[lo - (d0 - 1):hi - (d0 - 1)] = (
            ip[:, lo:hi].transpose(1, 0, 2, 3).reshape(hi - lo, C, HWC))
        wa4 = np.zeros((NPL, 128, 2, 128), np.float32)
        for i in range(NPL):
            dg = d0 + i
            dcase = 0 if dg == 0 else (2 if dg == D - 1 else 1)
            wa4[i, 0:64, 0, :] = wtap(0, 1, 1)
            wa4[i, 64:128, 0, :] = wtap(1, 1, 1)
            wa4[i, 0:9, 1, :] = corr_tab[dcase] * wsc[None, :]
        m = dict(common)
        m["inp_t"] = halo.astype(bf)
        m["inp_b"] = np.ascontiguousarray(
            ip[:, d0:d0 + NPL].transpose(1, 0, 2, 3)
            .reshape(NPL, C, HWC)).astype(bf)
        m["wa4"] = wa4.transpose(1, 0, 2, 3).astype(f8)
        m["sd8"] = (sdv[b] / wsc).reshape(128, 1).astype(np.float32)
        m["modb2"] = (mod_b + sdv[b] * base_mid * b1).reshape(
            128, 1).astype(np.float32)
        in_maps.append(m)
    return in_maps


def kernel(**inputs):
    from concourse.bass_utils import run_bass_kernel_spmd
    if "nc" not in _CACHE:
        _CACHE["nc"] = _build()
    nc = _CACHE["nc"]
    in_maps = _host_prep(inputs)
    res = run_bass_kernel_spmd(nc, in_maps, list(range(8)))
    _CACHE["last_res"] = res
    out = np.empty((2, C, D, H, W), np.float32)
    for k in range(8):
        b, d0 = k // 4, (k % 4) * NPL
        o = np.asarray(res.results[k]["out"]).astype(np.float32)
        out[b, :, d0:d0 + NPL] = o.reshape(NPL, C, H, W).transpose(1, 0, 2, 3)
    return out
